# revision 48
# baseline (speedup 1.0000x reference)
"""Trainium2 Bass kernel for DualAttention (position + channel attention).

Shapes (hardcoded): x (2, 512, 64, 64) fp32; wq/wk (64, 512); wv (512, 512).
Sharding: 8 cores = 2 batches x 4 chunks (chunk index = partition_id % 4).
Each core computes
  - position attention for a 1024-wide slice of the 4096 query positions
    (output transposed: (1024, 512) bf16, normalized, without the v-bias), and
  - channel attention for a 128-row slice of the 512 channels
    (output (128, 4096) bf16).
Host combines: out = a*gp*pos + b*gc*chan + (1+a+b)*x  (+ bv folded into pos).

Math notes (all-fp8 DoubleRow design, CPU-validated rel err ~8.7e-3):
  - softmax rows: row-constant terms cancel, so the k-bias is dropped and no
    max-subtraction is needed. P is stored fp8e5m2 as exp(S - 7.5); the global
    bias cancels in the P/rowsum normalization. Rowsums are computed on the PE
    as all-ones fp8 DR matmuls against the quantized P (self-consistent, and
    the [128,512] output is already partition-broadcast for the Z scaling).
  - Z = x8 @ P~^T runs fp8 DR; Z is then normalized by 1/rowsum and quantized
    to e4m3 (zn), and pos^T = zn^T @ wv8 runs fp8 DR too (wv in e4m3).
  - channel energy = 3-pass fp8 DR using the resident x^T fp8 stream as both
    stationary-hi and moving-hi: hi*hi + lo*hi + hi*lo, with e4m3 residuals
    (xtlo own rows, xtblo all channels). More accurate than a bf16 pass.
  - channel attention: exp(min - e) quantized e4m3, rowsum of the quantized
    weights (self-consistent), PE-transposed, chan = att8 @ x8 in fp8 DR with
    rows scaled by 1/rowsum at the end.
  - the host pre-rotates the position axis per core (slot s holds physical
    group (2*qt+s) % 8) and cyclically rotates the channel axis by qt*128 for
    the x^T / wv / xf8 streams so each core's own channel block is always
    0:128; host un-rotates the channel output. q/k projections use unrotated
    channel order (matching wkq).
  - phase order: L0 = q/k projections + S(0)/exp(0) (chunk-0 Z deferred so L0
    only needs the 4.2MB xfp stream); LB = S(1)/exp(1) + Z(0) kt0-2 + energy;
    tail = Z(0) kt3, rowsum(1), Z(1), pos^T(0), transposes, pos^T(1), chan.
  - exps run on [128,2,512] psum pair tiles (one ACT instruction per j-pair).
  - float32r on the PE for the S matmuls (k_sb/q_sb staged fp32).
"""

import numpy as np

B = 2
C = 512
D = 64          # C // 8
N = 4096        # h * w
NI = 1024       # query positions per core
CH = 128        # channel rows per core
NCORES = 8

NJT = N // 128    # 32 j-tiles
NKT = C // 128    # 4 contraction tiles over channels
NNT = N // 512    # 8 n-groups of 512
NPAIR = N // 256  # 16 j-tile pairs (DoubleRow)

PBIAS = 7.5       # global exp bias; cancels in normalization

_cache = {}


def _build():
    import concourse.bacc as bacc
    import concourse.mybir as mybir
    import concourse.tile as tile
    from concourse import bass as bass

    fp32 = mybir.dt.float32
    int32 = mybir.dt.int32
    bf16 = mybir.dt.bfloat16
    f32r = mybir.dt.float32r
    f8e4 = mybir.dt.float8e4
    f8e5 = mybir.dt.float8e5
    PSUM = bass.MemorySpace.PSUM
    DR = mybir.MatmulPerfMode.DoubleRow

    Exp = mybir.ActivationFunctionType.Exp
    Ident = mybir.ActivationFunctionType.Identity
    X = mybir.AxisListType.X
    amin = mybir.AluOpType.min
    aadd = mybir.AluOpType.add
    amult = mybir.AluOpType.mult

    nc = bacc.Bacc("TRN2", target_bir_lowering=False, debug=False)

    xfp_d = nc.dram_tensor("xfp", [128, NNT, NKT, 512], bf16, kind="ExternalInput")
    xt8_d = nc.dram_tensor("xt8", [128, NPAIR, 2, C], f8e4, kind="ExternalInput")
    xtlo_d = nc.dram_tensor("xtlo", [128, NPAIR, 2, CH], f8e4, kind="ExternalInput")
    xtblo_d = nc.dram_tensor("xtblo", [128, NPAIR, 2, C], f8e4, kind="ExternalInput")
    xf8_d = nc.dram_tensor("xf8", [128, 2, 2, N], f8e4, kind="ExternalInput")
    wkq_d = nc.dram_tensor("wkq", [128, NKT, 128], bf16, kind="ExternalInput")
    wvq_d = nc.dram_tensor("wvq8", [128, 2, 2, C], f8e4, kind="ExternalInput")
    b128_d = nc.dram_tensor("b128", [128, 1], fp32, kind="ExternalInput")
    ones8_d = nc.dram_tensor("ones8", [128, 2, 128], f8e4, kind="ExternalInput")
    id_d = nc.dram_tensor("ident", [128, 128], fp32, kind="ExternalInput")

    post_d = nc.dram_tensor("post", [NI, C], bf16, kind="ExternalOutput")
    chan_d = nc.dram_tensor("chan", [CH, N], bf16, kind="ExternalOutput")

    with tile.TileContext(nc) as tc:
        with (
            tc.tile_pool(name="const", bufs=1) as constp,
            tc.tile_pool(name="res", bufs=1) as resp,
            tc.tile_pool(name="wk", bufs=1) as workp,
            tc.tile_pool(name="cout", bufs=5) as coutp,
        ):
            # ---- constants + input streams (issue order = priority) ----
            # ACT issues no DMAs (its LoadActFuncSet would delay them);
            # xfr0 halves go out on the otherwise-idle Pool and DVE queues.
            wkq_sb = constp.tile([128, NKT, 128], bf16)
            nc.sync.dma_start(wkq_sb[:], wkq_d.ap())
            nb_sb = constp.tile([128, 1], fp32)
            nc.vector.memset(nb_sb[:], -PBIAS)
            wsrc_sb = constp.tile([128, 64], fp32)
            nc.vector.memset(wsrc_sb[:], 0.0)

            xfr = [
                resp.tile([128, NKT, 512], bf16, name=f"xfr{s}", tag=f"xfr{s}")
                for s in range(NNT)
            ]
            b128_sb = constp.tile([128, 1], fp32)
            nc.gpsimd.dma_start(b128_sb[:], b128_d.ap())
            nc.gpsimd.dma_start(xfr[0][:, 0:2], xfp_d.ap()[:, 0, 0:2])
            nc.gpsimd.dma_start(xfr[0][:, 2:4], xfp_d.ap()[:, 0, 2:4])
            for s in range(1, NNT):
                nc.sync.dma_start(xfr[s][:], xfp_d.ap()[:, s])
            ones8_sb = constp.tile([128, 2, 128], f8e4)
            nc.gpsimd.dma_start(ones8_sb[:], ones8_d.ap())
            id_sb = constp.tile([128, 128], fp32)
            nc.gpsimd.dma_start(id_sb[:], id_d.ap())

            # LB inputs: fp8 x^T stream + residuals, paced per slot
            xt8r = [
                resp.tile([128, 2, 2, C], f8e4, name=f"xt8r{s}", tag=f"xt8r{s}")
                for s in range(NNT)
            ]
            xtlo_sb = resp.tile([128, NPAIR, 2, CH], f8e4, name="xtlo_sb")
            xtblo_sb = resp.tile([128, NPAIR, 2, C], f8e4, name="xtblo_sb")
            for s in range(NNT):
                nc.sync.dma_start(xt8r[s][:], xt8_d.ap()[:, 2 * s : 2 * s + 2])
                nc.sync.dma_start(
                    xtlo_sb[:, 2 * s : 2 * s + 2], xtlo_d.ap()[:, 2 * s : 2 * s + 2]
                )
                nc.sync.dma_start(
                    xtblo_sb[:, 2 * s : 2 * s + 2], xtblo_d.ap()[:, 2 * s : 2 * s + 2]
                )
            # tail inputs
            wvq_sb = constp.tile([128, 2, 2, C], f8e4)
            nc.sync.dma_start(wvq_sb[:], wvq_d.ap())
            xf8_sb = resp.tile([128, 2, 2, N], f8e4, name="xf8_sb")
            nc.sync.dma_start(xf8_sb[:], xf8_d.ap())

            # staging for the DVE exp2 bit-hack (int-valued float -> bitcast)
            tint = [
                resp.tile([128, 2, 512], int32, name=f"tint{i}", tag=f"tint{i}")
                for i in range(2)
            ]
            L2E = 1.4426950408889634
            BH_MUL = L2E * (1 << 23)
            BH_ADD = (127.0 - PBIAS * L2E) * (1 << 23)


            k_sb = resp.tile([D, N], fp32, name="k_sb")
            q_sb = resp.tile([D, NI], fp32, name="q_sb")
            p8 = [
                [
                    resp.tile(
                        [128, 2, 512], f8e5, name=f"p8_{ic}_{p}", tag=f"p8_{ic}_{p}"
                    )
                    for p in range(NPAIR)
                ]
                for ic in range(2)
            ]
            zn8 = [
                resp.tile([128, NKT, 512], f8e4, name=f"zn8_{ic}", tag=f"zn8_{ic}")
                for ic in range(2)
            ]
            invr = [
                workp.tile([128, 512], fp32, tag=f"invr{ic}", name=f"invr{ic}")
                for ic in range(2)
            ]

            def emit_kproj(kqps, s):
                """k projection for slot s (wk = cols 64:128 of wkq)."""
                k_ps = kqps.tile([D, 512], fp32, tag="kq_ps", name=f"k_ps{s}")
                for kt in range(NKT):
                    nc.tensor.matmul(
                        k_ps[:],
                        wkq_sb[:, kt, 64:128],
                        xfr[s][:, kt, :],
                        start=(kt == 0),
                        stop=(kt == NKT - 1),
                    )
                nc.vector.tensor_copy(
                    k_sb[:, s * 512 : (s + 1) * 512].bitcast(f32r), k_ps[:]
                )

            def emit_qproj(kqps, ic):
                """q projection for chunk ic from resident xfr[ic]."""
                q_ps = kqps.tile([D, 512], fp32, tag="kq_ps", name=f"q_ps{ic}")
                for kt in range(NKT):
                    nc.tensor.matmul(
                        q_ps[:],
                        wkq_sb[:, kt, 0:64],
                        xfr[ic][:, kt, :],
                        start=(kt == 0),
                        stop=(kt == NKT - 1),
                    )
                nc.scalar.activation(
                    q_sb[:, ic * 512 : (ic + 1) * 512].bitcast(f32r),
                    q_ps[:],
                    Ident,
                    bias=b128_sb[0:64],
                    scale=1.0,
                )

            def emit_exp(sp, ic, p):
                """exp(S - PBIAS) -> p8 e5m2. Every 4th pair runs on the DVE
                via the exp2 bit hack (numerically free under e5m2), taking
                ~1us per pair off the critical ACT chain."""
                if ic == 0 and p in (13, 15):
                    ti = tint[(p // 2) % 2]
                    nc.vector.tensor_scalar(
                        ti[:], sp[:], BH_MUL, BH_ADD, amult, aadd
                    )
                    nc.vector.tensor_copy(p8[ic][p][:], ti[:].bitcast(fp32))
                else:
                    nc.scalar.activation(p8[ic][p][:], sp[:], Exp, bias=nb_sb[:])

            def emit_spair(sp, ic, p):
                """S matmuls for j-tile pair p of chunk ic into psum pair sp."""
                qs = q_sb[:, ic * 512 : (ic + 1) * 512].bitcast(f32r)
                for u in range(2):
                    jt = 2 * p + u
                    nc.tensor.matmul(
                        sp[:, u, :],
                        k_sb[:, jt * 128 : (jt + 1) * 128].bitcast(f32r),
                        qs,
                        start=True,
                        stop=True,
                    )

            # ---- L0: projections + S(0)/exp(0) + interleaved rowsum(0) ----
            # rowsum(0): all-ones fp8 DR against the quantized P (self-
            # consistent); the [128,512] result is partition-broadcast already.
            with (
                tc.tile_pool(name="s0", bufs=3, space=PSUM) as s0p,
                tc.tile_pool(name="rs0", bufs=1, space=PSUM) as rs0p,
            ):
                rs_ps = rs0p.tile([128, 512], fp32, tag="rs0")
                with tc.tile_pool(name="kq", bufs=1, space=PSUM) as kqp:
                    # warm the PE p-state while the first DMAs land: wide
                    # no-op matmuls (~400ns each) on a memset tile keep the
                    # PE continuously busy so the ramp reaches full speed
                    # before the projections start
                    warm = kqp.tile([D, 512], fp32, tag="kq_ps", name="warm")
                    for w in range(8):
                        nc.tensor.matmul(
                            warm[0:1, 0:64], nb_sb[:], wsrc_sb[:],
                            start=True, stop=True,
                        )
                    emit_qproj(kqp, 0)
                    emit_kproj(kqp, 0)
                    for p in range(NPAIR):
                        sp = s0p.tile([128, 2, 512], fp32, tag="s_ps", name=f"s0_{p}")
                        emit_spair(sp, 0, p)
                        if p % 2 == 0 and p // 2 + 1 < NNT:
                            emit_kproj(kqp, p // 2 + 1)
                        if p == 4:
                            emit_qproj(kqp, 1)
                        if p >= 3:
                            nc.tensor.matmul(
                                rs_ps[:], ones8_sb[:], p8[0][p - 3][:],
                                start=(p == 3), stop=False, perf_mode=DR,
                            )
                        emit_exp(sp, 0, p)
                for p in range(NPAIR - 3, NPAIR):
                    nc.tensor.matmul(
                        rs_ps[:], ones8_sb[:], p8[0][p][:],
                        start=False, stop=(p == NPAIR - 1), perf_mode=DR,
                    )
                nc.vector.reciprocal(invr[0][:], rs_ps[:])

            # ---- LB: S(1)/exp(1) + Z(0) kt0-2 + 3-pass fp8 energy ----
            with (
                tc.tile_pool(name="s1", bufs=2, space=PSUM) as s1p,
                tc.tile_pool(name="z0", bufs=3, space=PSUM) as z0p,
                tc.tile_pool(name="re", bufs=1, space=PSUM) as rep,
            ):
                z0_t = [
                    z0p.tile([128, 512], fp32, tag="z0", name=f"z0_{kt}")
                    for kt in range(3)
                ]
                r_ps = rep.tile([128, C], fp32, tag="r_ps")
                for p in range(NPAIR):
                    sp = s1p.tile([128, 2, 512], fp32, tag="s_ps1", name=f"s1_{p}")
                    emit_spair(sp, 1, p)
                    xts = xt8r[p // 2][:, p % 2]
                    for kt in range(3):
                        nc.tensor.matmul(
                            z0_t[kt][:],
                            xts[:, :, kt * 128 : (kt + 1) * 128],
                            p8[0][p][:],
                            start=(p == 0),
                            stop=(p == NPAIR - 1),
                            perf_mode=DR,
                        )
                    # energy: hi*hi + lo*hi + hi*lo (one DR accumulation group)
                    nc.tensor.matmul(
                        r_ps[:], xts[:, :, 0:CH], xts[:],
                        start=(p == 0), stop=False, perf_mode=DR,
                    )
                    nc.tensor.matmul(
                        r_ps[:], xtlo_sb[:, p], xts[:],
                        start=False, stop=False, perf_mode=DR,
                    )
                    nc.tensor.matmul(
                        r_ps[:], xts[:, :, 0:CH], xtblo_sb[:, p],
                        start=False, stop=(p == NPAIR - 1), perf_mode=DR,
                    )
                    emit_exp(sp, 1, p)

                # channel softmax first on DVE: the m_sb -> a8 chain is what
                # frees the r_ps bank for the tail z-pools
                m_sb = workp.tile([128, 1], fp32, tag="m_sb")
                nc.vector.tensor_reduce(m_sb[:], r_ps[:], axis=X, op=amin)
                a8 = workp.tile([128, C], f8e4, tag="a8")
                nc.scalar.activation(a8[:], r_ps[:], Exp, bias=m_sb[:], scale=-1.0)
                # chunk-0 zn for kt0-2 (z0_t stopped; invr[0] ready)
                for kt in range(3):
                    nc.vector.tensor_tensor(
                        zn8[0][:, kt, :], z0_t[kt][:], invr[0][:], op=amult
                    )
                a8f = workp.tile([128, C], fp32, tag="a8f")
                nc.vector.tensor_copy(a8f[:], a8[:])
                s_c = workp.tile([128, 1], fp32, tag="s_c")
                nc.vector.tensor_reduce(s_c[:], a8f[:], axis=X, op=aadd)
                invc = workp.tile([128, 1], fp32, tag="invc")
                nc.vector.reciprocal(invc[:], s_c[:])

            # ---- tail 1: Z(0) kt3, Z(1) kt-major + rowsum(1), posT(0) ----
            def emit_post(pool, ic, sl, tag):
                po = pool.tile([128, 512], fp32, tag=tag, name=f"po{ic}_{sl}")
                for kp in range(2):
                    nc.tensor.matmul(
                        po[:],
                        zn8[ic][:, 2 * kp : 2 * kp + 2, sl * 128 : (sl + 1) * 128],
                        wvq_sb[:, kp],
                        start=(kp == 0),
                        stop=(kp == 1),
                        perf_mode=DR,
                    )
                post_t = workp.tile(
                    [128, 512], bf16, tag="post", name="post_t", bufs=4
                )
                if sl % 2 == 0:
                    nc.scalar.copy(post_t[:], po[:])
                else:
                    nc.vector.tensor_copy(post_t[:], po[:])
                # post outputs stream from the idle Pool DGE so the sync
                # queue stays clear for the chan outputs
                nc.gpsimd.dma_start(
                    post_d.ap()[ic * 512 + sl * 128 : ic * 512 + (sl + 1) * 128, :],
                    post_t[:],
                )

            with (
                tc.tile_pool(name="zt", bufs=4, space=PSUM) as ztp,
                tc.tile_pool(name="rs1", bufs=1, space=PSUM) as rs1p,
                tc.tile_pool(name="out", bufs=3, space=PSUM) as outp,
            ):
                def emit_zgroup(zt, src8, zn_slice):
                    """one Z accumulation group over all 16 pairs + zn scale"""
                    for p in range(NPAIR):
                        nc.tensor.matmul(
                            zt[:],
                            xt8r[p // 2][:, p % 2, :, src8[0]:src8[1]],
                            p8[src8[2]][p][:],
                            start=(p == 0),
                            stop=(p == NPAIR - 1),
                            perf_mode=DR,
                        )

                def emit_chan(s):
                    c_ps = outp.tile([128, 512], fp32, tag="out", name=f"c{s}")
                    for kp in range(2):
                        nc.tensor.matmul(
                            c_ps[:],
                            at8[:, kp],
                            xf8_sb[:, kp, :, s * 512 : (s + 1) * 512],
                            start=(kp == 0),
                            stop=(kp == 1),
                            perf_mode=DR,
                        )
                    if s % 2 == 0:
                        co = coutp.tile([128, 2, 512], bf16, tag="cout")
                        nc.vector.tensor_scalar_mul(co[:, 0, :], c_ps[:], invc[:])
                        emit_chan.co = co
                    else:
                        co = emit_chan.co
                        nc.scalar.activation(
                            co[:, 1, :], c_ps[:], Ident, bias=0.0, scale=invc[:]
                        )
                    # last pair unmerged so the final DMA chain is short
                    if s == NNT - 2:
                        nc.sync.dma_start(
                            chan_d.ap()[:, s * 512 : (s + 1) * 512], co[:, 0, :]
                        )
                    elif s == NNT - 1:
                        nc.sync.dma_start(
                            chan_d.ap()[:, s * 512 : (s + 1) * 512], co[:, 1, :]
                        )
                    elif s % 2 == 1:
                        nc.sync.dma_start(
                            chan_d.ap()[:, (s - 1) * 512 : (s + 1) * 512], co[:]
                        )

                # Z(0) kt3 first so zn(0) completes early
                z0_t3 = ztp.tile([128, 512], fp32, tag="zt", name="z0_3")
                emit_zgroup(z0_t3, (3 * 128, 4 * 128, 0), None)
                nc.vector.tensor_tensor(
                    zn8[0][:, 3, :], z0_t3[:], invr[0][:], op=amult
                )
                # Z(1) kt0 with rowsum(1) interleaved so invr(1) is ready early
                z1_0 = ztp.tile([128, 512], fp32, tag="zt", name="z1_0")
                rs_ps1 = rs1p.tile([128, 512], fp32, tag="rs1")
                for p in range(NPAIR):
                    nc.tensor.matmul(
                        z1_0[:],
                        xt8r[p // 2][:, p % 2, :, 0:128],
                        p8[1][p][:],
                        start=(p == 0),
                        stop=(p == NPAIR - 1),
                        perf_mode=DR,
                    )
                    nc.tensor.matmul(
                        rs_ps1[:],
                        ones8_sb[:],
                        p8[1][p][:],
                        start=(p == 0),
                        stop=(p == NPAIR - 1),
                        perf_mode=DR,
                    )
                nc.vector.reciprocal(invr[1][:], rs_ps1[:])
                nc.vector.tensor_tensor(
                    zn8[1][:, 0, :], z1_0[:], invr[1][:], op=amult
                )
                # posT(0) halves + attention transposes overlap Z(1)
                emit_post(outp, 0, 0, "out")
                emit_post(outp, 0, 1, "out")
                at8 = workp.tile([128, 2, 2, CH], f8e4, tag="at8")
                for kt in range(NKT):
                    t_ps = rs1p.tile([128, CH], fp32, tag="rs1", name="t_ps")
                    nc.tensor.transpose(
                        t_ps[:], a8f[:, kt * 128 : (kt + 1) * 128], id_sb[:]
                    )
                    if kt % 2 == 0:
                        nc.scalar.copy(at8[:, kt // 2, kt % 2, :], t_ps[:])
                    else:
                        nc.vector.tensor_copy(at8[:, kt // 2, kt % 2, :], t_ps[:])

                z1_1 = ztp.tile([128, 512], fp32, tag="zt", name="z1_1")
                emit_zgroup(z1_1, (128, 256, 1), None)
                nc.vector.tensor_tensor(
                    zn8[1][:, 1, :], z1_1[:], invr[1][:], op=amult
                )
                emit_chan(0)
                emit_chan(1)
                emit_post(outp, 0, 2, "out")
                z1_2 = ztp.tile([128, 512], fp32, tag="zt", name="z1_2")
                emit_zgroup(z1_2, (256, 384, 1), None)
                nc.vector.tensor_tensor(
                    zn8[1][:, 2, :], z1_2[:], invr[1][:], op=amult
                )
                emit_chan(2)
                emit_chan(3)
                emit_post(outp, 0, 3, "out")
                z1_3 = ztp.tile([128, 512], fp32, tag="zt", name="z1_3")
                emit_zgroup(z1_3, (384, 512, 1), None)
                nc.vector.tensor_tensor(
                    zn8[1][:, 3, :], z1_3[:], invr[1][:], op=amult
                )
                emit_chan(4)
                emit_chan(5)
                emit_chan(6)
                emit_chan(7)
                for sl in range(3):
                    emit_post(outp, 1, sl, "out")
                # last output: copy halves on both engines, DMAs on two queues
                po = outp.tile([128, 512], fp32, tag="out", name="po1_3")
                for kp in range(2):
                    nc.tensor.matmul(
                        po[:],
                        zn8[1][:, 2 * kp : 2 * kp + 2, 3 * 128 : 4 * 128],
                        wvq_sb[:, kp],
                        start=(kp == 0),
                        stop=(kp == 1),
                        perf_mode=DR,
                    )
                post_t = workp.tile(
                    [128, 512], bf16, tag="post", name="post_t", bufs=4
                )
                nc.scalar.copy(post_t[:, 0:256], po[:, 0:256])
                nc.vector.tensor_copy(post_t[:, 256:512], po[:, 256:512])
                nc.gpsimd.dma_start(
                    post_d.ap()[512 + 3 * 128 : 1024, 0:256], post_t[:, 0:256]
                )
                nc.sync.dma_start(
                    post_d.ap()[512 + 3 * 128 : 1024, 256:512], post_t[:, 256:512]
                )

    nc.compile()
    return nc


def _get_nc():
    if "nc" not in _cache:
        _cache["nc"] = _build()
    return _cache["nc"]


def make_in_maps(x, wq, bq, wk, bk, wv, bv):
    """Build the 8 per-core input dicts from full inputs (host-prepacked)."""
    import ml_dtypes

    e4 = ml_dtypes.float8_e4m3
    bfl = ml_dtypes.bfloat16

    xr = np.ascontiguousarray(x.reshape(B, C, N)).astype(np.float32)
    ident = np.eye(128, dtype=np.float32)
    ones8 = np.ones((128, 2, 128), dtype=e4)
    # fused [wq.T | wk.T] -> [128, NKT, 128] bf16
    wkq = np.hstack([wq.T, wk.T]).astype(bfl)          # (C, 128)
    wkq = np.ascontiguousarray(wkq.reshape(NKT, 128, 128).transpose(1, 0, 2))
    wvT = np.ascontiguousarray(wv.T).astype(np.float32)  # (cin, cout)
    b128 = np.zeros((128, 1), np.float32)
    b128[:D, 0] = np.asarray(bq, np.float32)

    in_maps = []
    for b in range(B):
        xf = xr[b]                                    # (C, N)
        xfb = xf.astype(bfl)
        # [p, g, kt, n'] layout (position groups unrotated, channels unrotated)
        xfp_base = xfb.reshape(NKT, 128, NNT, 512).transpose(1, 2, 0, 3)
        # x^T in [128, NNT, NKT, C] fp32 (position-partitioned)
        xtq_base = (
            np.ascontiguousarray(xf.T).reshape(NNT, NKT, 128, C).transpose(2, 0, 1, 3)
        )
        for qt in range(4):
            rot = [(2 * qt + s) % NNT for s in range(NNT)]
            crot = [(qt * CH + c) % C for c in range(C)]
            xtq_rot = np.ascontiguousarray(xtq_base[:, rot][:, :, :, crot])
            xtq_pairs = xtq_rot.reshape(128, NPAIR, 2, C)
            xt8 = xtq_pairs.astype(e4)
            resid = xtq_pairs - xt8.astype(np.float32)
            # chan-rotated x in [c, n] for the chan matmul, position-rotated
            xn = xf[crot].reshape(C, NNT, 512)[:, rot].reshape(C, N)
            xf8 = np.ascontiguousarray(
                xn.astype(e4).reshape(2, 2, 128, N).transpose(2, 0, 1, 3)
            )
            wvq8 = np.ascontiguousarray(
                wvT[crot].reshape(2, 2, 128, C).transpose(2, 0, 1, 3).astype(e4)
            )
            in_maps.append(
                {
                    "xfp": np.ascontiguousarray(xfp_base[:, rot]),
                    "xt8": np.ascontiguousarray(xt8),
                    "xtlo": np.ascontiguousarray(resid[:, :, :, 0:CH].astype(e4)),
                    "xtblo": np.ascontiguousarray(resid.astype(e4)),
                    "xf8": xf8,
                    "wkq": wkq,
                    "wvq8": wvq8,
                    "b128": b128,
                    "ones8": ones8,
                    "ident": ident,
                }
            )
    return in_maps


def assemble(results, x, bv, gamma_pos, gamma_chan, alpha, beta):
    """Combine per-core outputs into the full module output."""
    xr = x.reshape(B, C, N)
    a = float(np.asarray(alpha).reshape(-1)[0])
    be = float(np.asarray(beta).reshape(-1)[0])
    gp = float(np.asarray(gamma_pos).reshape(-1)[0])
    gc = float(np.asarray(gamma_chan).reshape(-1)[0])
    out = np.empty((B, C, N), dtype=np.float32)
    for b in range(B):
        posT = np.concatenate(
            [np.asarray(results[b * 4 + qt]["post"], np.float32) for qt in range(4)],
            axis=0,
        )  # (N, C)
        pos = posT.T + bv.reshape(C, 1)
        chan = np.empty((C, N), np.float32)
        for qt in range(4):
            cres = np.asarray(results[b * 4 + qt]["chan"], np.float32)  # (CH, N)
            for s in range(NNT):
                g = (2 * qt + s) % NNT
                chan[qt * CH : (qt + 1) * CH, g * 512 : (g + 1) * 512] = cres[
                    :, s * 512 : (s + 1) * 512
                ]
        out[b] = a * gp * pos + be * gc * chan + (1.0 + a + be) * xr[b]
    return out.reshape(B, C, 64, 64)


def kernel(x, wq, bq, wk, bk, wv, bv, gamma_pos, gamma_chan, alpha, beta):
    from concourse import bass_utils

    # accept jax or numpy inputs
    x = np.asarray(x, np.float32)
    wq = np.asarray(wq, np.float32)
    bq = np.asarray(bq, np.float32)
    wk = np.asarray(wk, np.float32)
    wv = np.asarray(wv, np.float32)
    bv = np.asarray(bv, np.float32)

    nc = _get_nc()
    in_maps = make_in_maps(x, wq, bq, wk, bk, wv, bv)
    res = bass_utils.run_bass_kernel_spmd(nc, in_maps, core_ids=list(range(NCORES)))
    return assemble(res.results, x, bv, gamma_pos, gamma_chan, alpha, beta)
